# revision 4
# baseline (speedup 1.0000x reference)
"""Trainium2 Bass kernel for nn_CustomLoss_21784074125724.

loss = mean_b sqrt(sum_d (output[b,d] - label[b,d])^2)   with B=16, D=2097152.

Sharding: data-parallel over the batch dim — each of the 8 cores takes 2
samples (2 x 8 MB per input tensor). Per core the kernel streams chunks of
[128, CHUNK] f32 through SBUF: DVE computes diff = a - b, ACT computes
Square(diff) with a fused per-partition free-dim accumulation into one
column of a [128, n_chunks] stats tile. The tiny final reduction
(128 x n_chunks values per core), sqrt, and batch mean run on the host in
float64 — equivalent to the "tiny all-reduce" in the sharding hint.
"""

import sys

import numpy as np

for _p in ("/opt/trn_rl_repo", "/opt/trn_rl_repo/concourse"):
    if _p not in sys.path:
        sys.path.insert(0, _p)

import concourse.bacc as bacc
import concourse.bass as bass
import concourse.mybir as mybir
from concourse import tile
from concourse.bass_utils import run_bass_kernel_spmd

B = 16
D = 2097152
N_CORES = 8
S = B // N_CORES          # samples per core = 2
P = 128                   # SBUF partitions
FREE = D // P             # 16384 f32 per partition per sample
CHUNK = 4096              # free-dim chunk (2 MiB per DMA)
N_CHUNKS = FREE // CHUNK  # 4
N_COLS = S * N_CHUNKS     # 8 accumulator columns

_NC = None


def _build():
    global _NC
    if _NC is not None:
        return _NC

    nc = bacc.Bacc(
        "TRN2",
        target_bir_lowering=False,
        debug=False,
        enable_asserts=False,
    )
    out_d = nc.dram_tensor("output", [S, D], mybir.dt.float32, kind="ExternalInput").ap()
    lab_d = nc.dram_tensor("label", [S, D], mybir.dt.float32, kind="ExternalInput").ap()
    stats_d = nc.dram_tensor("stats", [P, N_COLS], mybir.dt.float32, kind="ExternalOutput").ap()

    # sample s's 8 MB row -> [128 partitions, 16384], each partition contiguous
    out_r = out_d.rearrange("s (p f) -> p s f", p=P)
    lab_r = lab_d.rearrange("s (p f) -> p s f", p=P)

    with tile.TileContext(nc) as tc:
        with (
            tc.tile_pool(name="a", bufs=3) as a_pool,
            tc.tile_pool(name="b", bufs=3) as b_pool,
            tc.tile_pool(name="d", bufs=2) as d_pool,
            tc.tile_pool(name="sq", bufs=2) as sq_pool,
            tc.tile_pool(name="st", bufs=1) as st_pool,
        ):
            stats = st_pool.tile([P, N_COLS], mybir.dt.float32)
            for s in range(S):
                for c in range(N_CHUNKS):
                    col = s * N_CHUNKS + c
                    sl = slice(c * CHUNK, (c + 1) * CHUNK)
                    a = a_pool.tile([P, CHUNK], mybir.dt.float32)
                    b = b_pool.tile([P, CHUNK], mybir.dt.float32)
                    nc.sync.dma_start(a[:], out_r[:, s, sl])
                    nc.sync.dma_start(b[:], lab_r[:, s, sl])
                    d = d_pool.tile([P, CHUNK], mybir.dt.float32)
                    nc.vector.tensor_sub(d[:], a[:], b[:])
                    sq = sq_pool.tile([P, CHUNK], mybir.dt.float32)
                    nc.scalar.activation(
                        sq[:],
                        d[:],
                        mybir.ActivationFunctionType.Square,
                        accum_out=stats[:, col : col + 1],
                    )
            nc.sync.dma_start(stats_d[:], stats[:])

    nc.compile()
    _NC = nc
    return nc


def _run(in_maps, **kwargs):
    nc = _build()
    return run_bass_kernel_spmd(nc, in_maps, core_ids=list(range(N_CORES)), **kwargs)


def _make_in_maps(output, label):
    output = np.asarray(output, dtype=np.float32)
    label = np.asarray(label, dtype=np.float32)
    assert output.shape == (B, D) and label.shape == (B, D)
    maps = []
    for i in range(N_CORES):
        sl = slice(i * S, (i + 1) * S)
        maps.append(
            {
                "output": np.ascontiguousarray(output[sl]),
                "label": np.ascontiguousarray(label[sl]),
            }
        )
    return maps


def _finish(results):
    dists = []
    for i in range(N_CORES):
        st = results[i]["stats"].astype(np.float64)
        for s in range(S):
            ss = st[:, s * N_CHUNKS : (s + 1) * N_CHUNKS].sum()
            dists.append(np.sqrt(ss))
    return np.float32(np.mean(dists))


def kernel(output, label):
    res = _run(_make_in_maps(output, label))
    return _finish(res.results)


def kernel_traced(output, label, **kwargs):
    """Like kernel() but returns (loss, BassKernelResults) with trace=True."""
    res = _run(_make_in_maps(output, label), trace=True, **kwargs)
    return _finish(res.results), res


# revision 5
# speedup vs baseline: 1.0080x; 1.0080x over previous
"""Trainium2 Bass kernel for nn_CustomLoss_21784074125724.

loss = mean_b sqrt(sum_d (output[b,d] - label[b,d])^2)   with B=16, D=2097152.

Sharding: data-parallel over the batch dim — each of the 8 cores takes 2
samples. Host packs output+label shards into one [2, S, D] DRAM tensor so
each chunk needs a single DMA. Per chunk the DVE computes diff = a - b
in place over the packed tile, then ACT computes Square(diff) with a fused
per-partition free-dim accumulation into one column of a [128, n_chunks]
stats tile. Chunk sizes descend toward the end of the stream so the
post-last-DMA compute tail is short. The tiny final reduction
(128 x n_chunks values per core), sqrt, and batch mean run on the host in
float64 — the "tiny all-reduce" of the sharding hint.
"""

import sys

import numpy as np

for _p in ("/opt/trn_rl_repo", "/opt/trn_rl_repo/concourse"):
    if _p not in sys.path:
        sys.path.insert(0, _p)

import concourse.bacc as bacc
import concourse.bass as bass
import concourse.mybir as mybir
from concourse import tile
from concourse.bass_utils import run_bass_kernel_spmd

B = 16
D = 2097152
N_CORES = 8
S = B // N_CORES          # samples per core = 2
P = 128                   # SBUF partitions
FREE = D // P             # 16384 f32 per partition per sample

# Free-dim chunking per sample. The last sample's stream ends with small
# chunks so the final DVE+ACT tail after the last input DMA is ~1 us
# instead of ~8 us.
CHUNKS_BODY = [4096, 4096, 4096, 4096]
CHUNKS_TAIL = [4096, 4096, 4096, 2048, 1024, 512, 512]
assert sum(CHUNKS_BODY) == FREE and sum(CHUNKS_TAIL) == FREE
CHUNK_PLAN = [CHUNKS_BODY] * (S - 1) + [CHUNKS_TAIL]
N_COLS = [len(p) for p in CHUNK_PLAN]
MAX_CHUNK = max(max(p) for p in CHUNK_PLAN)

_NC = None


def _build():
    global _NC
    if _NC is not None:
        return _NC

    nc = bacc.Bacc(
        "TRN2",
        target_bir_lowering=False,
        debug=False,
        enable_asserts=False,
    )
    packed_d = nc.dram_tensor(
        "packed", [2, S, D], mybir.dt.float32, kind="ExternalInput"
    ).ap()
    stats_ds = [
        nc.dram_tensor(
            f"stats{s}", [P, N_COLS[s]], mybir.dt.float32, kind="ExternalOutput"
        ).ap()
        for s in range(S)
    ]

    # [2, S, D] -> [p, t, s, f]: per (t, s) an 8 MB row seen as [128, 16384]
    packed_r = packed_d.rearrange("t s (p f) -> p t s f", p=P)

    with tile.TileContext(nc) as tc:
        with (
            tc.tile_pool(name="ab", bufs=4) as ab_pool,
            tc.tile_pool(name="sq", bufs=2) as sq_pool,
            tc.tile_pool(name="st", bufs=1) as st_pool,
        ):
            for s in range(S):
                stats = st_pool.tile([P, N_COLS[s]], mybir.dt.float32, tag=f"st{s}")
                off = 0
                for c, n in enumerate(CHUNK_PLAN[s]):
                    sl = slice(off, off + n)
                    off += n
                    ab = ab_pool.tile([P, 2, MAX_CHUNK], mybir.dt.float32)
                    nc.sync.dma_start(ab[:, :, :n], packed_r[:, :, s, sl])
                    # diff in place over the "output" half of the packed tile
                    nc.vector.tensor_sub(ab[:, 0, :n], ab[:, 0, :n], ab[:, 1, :n])
                    sq = sq_pool.tile([P, MAX_CHUNK], mybir.dt.float32)
                    nc.scalar.activation(
                        sq[:, :n],
                        ab[:, 0, :n],
                        mybir.ActivationFunctionType.Square,
                        accum_out=stats[:, c : c + 1],
                    )
                # sample 0's stats flush mid-kernel; the last sample's at the end
                nc.sync.dma_start(stats_ds[s][:], stats[:])

    nc.compile()
    _NC = nc
    return nc


def _run(in_maps, **kwargs):
    nc = _build()
    return run_bass_kernel_spmd(nc, in_maps, core_ids=list(range(N_CORES)), **kwargs)


def _make_in_maps(output, label):
    output = np.asarray(output, dtype=np.float32)
    label = np.asarray(label, dtype=np.float32)
    assert output.shape == (B, D) and label.shape == (B, D)
    maps = []
    for i in range(N_CORES):
        sl = slice(i * S, (i + 1) * S)
        packed = np.empty((2, S, D), dtype=np.float32)
        packed[0] = output[sl]
        packed[1] = label[sl]
        maps.append({"packed": packed})
    return maps


def _finish(results):
    dists = []
    for i in range(N_CORES):
        for s in range(S):
            ss = results[i][f"stats{s}"].astype(np.float64).sum()
            dists.append(np.sqrt(ss))
    return np.float32(np.mean(dists))


def kernel(output, label):
    res = _run(_make_in_maps(output, label))
    return _finish(res.results)


def kernel_traced(output, label, **kwargs):
    """Like kernel() but returns (loss, BassKernelResults) with trace=True."""
    res = _run(_make_in_maps(output, label), trace=True, **kwargs)
    return _finish(res.results), res


# revision 6
# speedup vs baseline: 1.0231x; 1.0150x over previous
"""Trainium2 Bass kernel for nn_CustomLoss_21784074125724.

loss = mean_b sqrt(sum_d (output[b,d] - label[b,d])^2)   with B=16, D=2097152.

Sharding: data-parallel over the batch dim — each of the 8 cores takes 2
samples. The host packs the two input tensors into one flat DRAM buffer,
interleaved at chunk granularity, so every chunk is a single DMA whose
per-partition source is one contiguous 2*chunk*4-byte segment (the best
descriptor shape). Per chunk: DVE computes diff = a - b in place over the
"a" half of the tile, then ACT computes Square(diff) into the dead "b"
half with a fused per-partition free-dim accumulation into one column of a
[128, n_chunks] stats tile. Chunk sizes descend toward the end of the
stream so the post-last-DMA compute tail is ~1 us. The tiny final
reduction, sqrt, and batch mean run on the host in float64 — the "tiny
all-reduce" of the sharding hint.
"""

import sys

import numpy as np

for _p in ("/opt/trn_rl_repo", "/opt/trn_rl_repo/concourse"):
    if _p not in sys.path:
        sys.path.insert(0, _p)

import concourse.bacc as bacc
import concourse.bass as bass
import concourse.mybir as mybir
from concourse import tile
from concourse.bass_utils import run_bass_kernel_spmd

B = 16
D = 2097152
N_CORES = 8
S = B // N_CORES          # samples per core = 2
P = 128                   # SBUF partitions
FREE = D // P             # 16384 f32 per partition per sample
TOTAL = 2 * S * D         # packed f32 elements per core

# Free-dim chunking per sample. The last sample's stream ends with small
# chunks so the final DVE+ACT tail after the last input DMA is short.
CHUNKS_BODY = [4096, 4096, 4096, 4096]
CHUNKS_TAIL = [4096, 4096, 4096, 2048, 1024, 512, 512]
assert sum(CHUNKS_BODY) == FREE and sum(CHUNKS_TAIL) == FREE
CHUNK_PLAN = [CHUNKS_BODY] * (S - 1) + [CHUNKS_TAIL]
N_COLS = [len(p) for p in CHUNK_PLAN]
MAX_CHUNK = max(max(p) for p in CHUNK_PLAN)

_NC = None


def _build():
    global _NC
    if _NC is not None:
        return _NC

    nc = bacc.Bacc(
        "TRN2",
        target_bir_lowering=False,
        debug=False,
        enable_asserts=False,
    )
    packed_d = nc.dram_tensor(
        "packed", [TOTAL], mybir.dt.float32, kind="ExternalInput"
    ).ap()
    stats_ds = [
        nc.dram_tensor(
            f"stats{s}", [P, N_COLS[s]], mybir.dt.float32, kind="ExternalOutput"
        ).ap()
        for s in range(S)
    ]

    with tile.TileContext(nc) as tc:
        with (
            tc.tile_pool(name="ab", bufs=5) as ab_pool,
            tc.tile_pool(name="st", bufs=1) as st_pool,
        ):
            off = 0
            for s in range(S):
                stats = st_pool.tile([P, N_COLS[s]], mybir.dt.float32, tag=f"st{s}")
                for c, n in enumerate(CHUNK_PLAN[s]):
                    src = packed_d[off : off + P * 2 * n].rearrange("(p x) -> p x", p=P)
                    off += P * 2 * n
                    ab = ab_pool.tile([P, 2 * MAX_CHUNK], mybir.dt.float32)
                    nc.sync.dma_start(ab[:, : 2 * n], src)
                    # diff in place over the "a" half
                    nc.vector.tensor_sub(ab[:, :n], ab[:, :n], ab[:, n : 2 * n])
                    # square into the dead "b" half; accumulate per partition
                    nc.scalar.activation(
                        ab[:, n : 2 * n],
                        ab[:, :n],
                        mybir.ActivationFunctionType.Square,
                        accum_out=stats[:, c : c + 1],
                    )
                # sample 0's stats flush mid-kernel; the last sample's at the end
                nc.sync.dma_start(stats_ds[s][:], stats[:])

    nc.compile()
    _NC = nc
    return nc


def _run(in_maps, **kwargs):
    nc = _build()
    return run_bass_kernel_spmd(nc, in_maps, core_ids=list(range(N_CORES)), **kwargs)


def _pack_core(output, label):
    """Interleave one core's shards chunk-wise into the flat DMA layout."""
    packed = np.empty(TOTAL, dtype=np.float32)
    off = 0
    for s in range(S):
        a = output[s].reshape(P, FREE)
        b = label[s].reshape(P, FREE)
        col = 0
        for n in CHUNK_PLAN[s]:
            blk = packed[off : off + P * 2 * n].reshape(P, 2, n)
            blk[:, 0, :] = a[:, col : col + n]
            blk[:, 1, :] = b[:, col : col + n]
            col += n
            off += P * 2 * n
    return packed


def _make_in_maps(output, label):
    output = np.asarray(output, dtype=np.float32)
    label = np.asarray(label, dtype=np.float32)
    assert output.shape == (B, D) and label.shape == (B, D)
    maps = []
    for i in range(N_CORES):
        sl = slice(i * S, (i + 1) * S)
        maps.append({"packed": _pack_core(output[sl], label[sl])})
    return maps


def _finish(results):
    dists = []
    for i in range(N_CORES):
        for s in range(S):
            ss = results[i][f"stats{s}"].astype(np.float64).sum()
            dists.append(np.sqrt(ss))
    return np.float32(np.mean(dists))


def kernel(output, label):
    res = _run(_make_in_maps(output, label))
    return _finish(res.results)


def kernel_traced(output, label, **kwargs):
    """Like kernel() but returns (loss, BassKernelResults) with trace=True."""
    res = _run(_make_in_maps(output, label), trace=True, **kwargs)
    return _finish(res.results), res


# revision 8
# speedup vs baseline: 1.0581x; 1.0342x over previous
"""Trainium2 Bass kernel for nn_CustomLoss_21784074125724.

loss = mean_b sqrt(sum_d (output[b,d] - label[b,d])^2)   with B=16, D=2097152.

Sharding: data-parallel over the batch dim — each of the 8 cores takes 2
samples. The host packs the two input tensors into one flat DRAM buffer,
interleaved at chunk granularity, so every chunk is a single DMA whose
per-partition source is one contiguous 2*chunk*4-byte segment (the best
descriptor shape). Per chunk: DVE computes diff = a - b in place over the
"a" half of the tile, then ACT computes Square(diff) into the dead "b"
half with a fused per-partition free-dim accumulation into one column of a
[128, n_chunks] stats tile. Chunk sizes descend toward the end of the
stream so the post-last-DMA compute tail is ~1 us. The tiny final
reduction, sqrt, and batch mean run on the host in float64 — the "tiny
all-reduce" of the sharding hint.
"""

import sys

import numpy as np

for _p in ("/opt/trn_rl_repo", "/opt/trn_rl_repo/concourse"):
    if _p not in sys.path:
        sys.path.insert(0, _p)

import concourse.bacc as bacc
import concourse.bass as bass
import concourse.mybir as mybir
from concourse import tile
from concourse.bass_utils import run_bass_kernel_spmd

B = 16
D = 2097152
N_CORES = 8
S = B // N_CORES          # samples per core = 2
P = 128                   # SBUF partitions
FREE = D // P             # 16384 f32 per partition per sample
TOTAL = 2 * S * D         # packed f32 elements per core

# Free-dim chunking per sample. The last sample's stream ends with small
# chunks so the final DVE+ACT tail after the last input DMA is short.
CHUNKS_BODY = [4096, 4096, 4096, 4096]
CHUNKS_TAIL = [4096, 4096, 4096, 2048, 1024, 512, 384, 128]
assert sum(CHUNKS_BODY) == FREE and sum(CHUNKS_TAIL) == FREE
CHUNK_PLAN = [CHUNKS_BODY] * (S - 1) + [CHUNKS_TAIL]
N_COLS = [len(p) for p in CHUNK_PLAN]
MAX_CHUNK = max(max(p) for p in CHUNK_PLAN)

_NC = None


def _build():
    global _NC
    if _NC is not None:
        return _NC

    nc = bacc.Bacc(
        "TRN2",
        target_bir_lowering=False,
        debug=False,
        enable_asserts=False,
    )
    packed_d = nc.dram_tensor(
        "packed", [TOTAL], mybir.dt.float32, kind="ExternalInput"
    ).ap()
    stats_ds = [
        nc.dram_tensor(
            f"stats{s}", [P, N_COLS[s]], mybir.dt.float32, kind="ExternalOutput"
        ).ap()
        for s in range(S)
    ]

    with tile.TileContext(nc) as tc:
        with (
            tc.tile_pool(name="ab", bufs=5) as ab_pool,
            tc.tile_pool(name="st", bufs=1) as st_pool,
        ):
            off = 0
            for s in range(S):
                stats = st_pool.tile([P, N_COLS[s]], mybir.dt.float32, tag=f"st{s}")
                for c, n in enumerate(CHUNK_PLAN[s]):
                    src = packed_d[off : off + P * 2 * n].rearrange("(p x) -> p x", p=P)
                    off += P * 2 * n
                    ab = ab_pool.tile([P, 2 * MAX_CHUNK], mybir.dt.float32)
                    nc.sync.dma_start(ab[:, : 2 * n], src)
                    # diff in place over the "a" half
                    nc.vector.tensor_sub(ab[:, :n], ab[:, :n], ab[:, n : 2 * n])
                    # square into the dead "b" half; accumulate per partition
                    nc.scalar.activation(
                        ab[:, n : 2 * n],
                        ab[:, :n],
                        mybir.ActivationFunctionType.Square,
                        accum_out=stats[:, c : c + 1],
                    )
                # stats DMA issues from the ACT sequencer's own HWDGE ring:
                # its wait is satisfied by ACT program order, so it never
                # stalls the Sync FIFO that feeds the input-chunk DMAs.
                nc.scalar.dma_start(stats_ds[s][:], stats[:])

    nc.compile()
    _NC = nc
    return nc


def _run(in_maps, **kwargs):
    nc = _build()
    return run_bass_kernel_spmd(nc, in_maps, core_ids=list(range(N_CORES)), **kwargs)


def _pack_core(output, label):
    """Interleave one core's shards chunk-wise into the flat DMA layout."""
    packed = np.empty(TOTAL, dtype=np.float32)
    off = 0
    for s in range(S):
        a = output[s].reshape(P, FREE)
        b = label[s].reshape(P, FREE)
        col = 0
        for n in CHUNK_PLAN[s]:
            blk = packed[off : off + P * 2 * n].reshape(P, 2, n)
            blk[:, 0, :] = a[:, col : col + n]
            blk[:, 1, :] = b[:, col : col + n]
            col += n
            off += P * 2 * n
    return packed


def _make_in_maps(output, label):
    output = np.asarray(output, dtype=np.float32)
    label = np.asarray(label, dtype=np.float32)
    assert output.shape == (B, D) and label.shape == (B, D)
    maps = []
    for i in range(N_CORES):
        sl = slice(i * S, (i + 1) * S)
        maps.append({"packed": _pack_core(output[sl], label[sl])})
    return maps


def _finish(results):
    dists = []
    for i in range(N_CORES):
        for s in range(S):
            ss = results[i][f"stats{s}"].astype(np.float64).sum()
            dists.append(np.sqrt(ss))
    return np.float32(np.mean(dists))


def kernel(output, label):
    res = _run(_make_in_maps(output, label))
    return _finish(res.results)


def kernel_traced(output, label, **kwargs):
    """Like kernel() but returns (loss, BassKernelResults) with trace=True."""
    res = _run(_make_in_maps(output, label), trace=True, **kwargs)
    return _finish(res.results), res
